# revision 73
# baseline (speedup 1.0000x reference)
"""Multi-head self-attention TRN2 kernel (16 heads, D=1024, x:[2,2048,1024]).

Sharding: 8 cores = 2 (batch) x 4 (head groups of 4 heads).  Host sums the
4 partials per batch and adds bo' = bo + bv@wo (bv passes through softmax
unchanged, bk cancels in softmax normalization, so neither is on-device).

Per-core structure:
  front: DMAs spread over the SP/Pool/ACT queues so wq + xT(n0) + bq land
  first (the DMA bus serializes transfers, so first-needed pieces lead
  each queue and the first projections consume chunks in bus-arrival
  order); PE p-state warms on f32 dummy matmuls until data arrives.
  qT/kT = (x_b @ wq + bq)^T head-dim-major             [256, 2048] f32r
  v     = x_b @ wv, token-major, ones-augmented         [2048,4,65] bf16
  scores: kT-stationary matmuls [k=128chunk, q=512] f32r PSUM, emitted
    with a scheduler priority boost so ready scores preempt fillers on
    the PE (the ACT exp stream is the pacer and must never starve)
  exp via ACT (scale=1/8, zero-bias AP, no max subtraction) -> bf16 SBUF
  AV q-stationary: lhsT=exp[128k,128q], rhs=v[128k,65] bf16, out [128q,65]
    accumulated with start=False onto DVE-zeroed PSUM
  normalize: DVE reciprocal of sums column + broadcast multiply -> opair
  PE transpose (vs bf16 identity) -> oT bf16 via DVE 2x-mode copies
  out = oT^T @ wo(bf16) per 128-token chunk; output copies on DVE, out
  DMAs on SP.

Pipeline: 8 windows = (pair0 n0..n3, pair1 n0..n3), window pace set by ACT
(16 exps/window, ~16.6us).  Window w computes scores+exp for its own
512-q chunk while draining window w-1's AV.  w0-w4: 1 AV group/slot +
projection fillers (proj blocks staged in 2-slot halves so a PE burst
never outruns the 2-deep psc cushion; v blocks sit early in w0/w1 --
their drains consume them at w1 slot t/2, so late placement stalls the
drain pipeline).  w5-w7: 2 AV groups/slot over slots 0-3, normalize prev
at slot 3, transposes slot 4, output projection slots 4-6; w7 drains its
own groups 0-6 at slots 5-7 after its avs take over the psum ring at
slot 4 (ring handoff order w6-avs -> tps -> own-avs is load-bearing: a
tile allocated after the own-avs would wait out their whole lifetime).
Tail is per-qsub software-pipelined: drain g7(q+2) and normalize(q+1)
are emitted before transpose(q)/oproj(q) so the in-order PE queue never
head-of-line blocks on DVE latency; tail transposes live in the retired
scores ring; late-tail oproj tiles alternate the acc/av psum rings; the
kernel-ending piece (tok15 nn1) runs first of its pair with its copy on
DVE and its DMA on the by-then-idle ACT queue.

TimelineSim: 162464 ns (inherited baseline: 167283; rel err 4.6e-3 on HW).
PE busy 143.8us (floor ~142: proj 41 + scores 54.6 + AV 27.7 + oproj 13.7
+ transposes 1.7 + warmup), ACT 136.6us (exp floor ~133).  fp8 DoubleRow
(0.5 cy/row) measurably breaks the 2e-2 gate (4.7-16% rel err), gpsimd
cannot read PSUM (walrus rejects), and PSUM->DRAM DMA is disallowed, so
both engines sit at their bf16/f32r floors; the residual ~12us over the
zero-idle bound is the w0-w2 projection-capacity hump (drain-lag and
filler-reorder restructures regressed four times), the serial-DMA-bus
front (first exp is wk-bus-bound), and the fixed end DMA+barrier chain.
"""

import os
import sys
from contextlib import ExitStack

import numpy as np

for _p in ("/opt/trn_rl_repo", os.path.expanduser("~/.axon_site/_ro/trn_rl_repo")):
    if os.path.isdir(_p) and _p not in sys.path:
        sys.path.insert(0, _p)

import concourse.bass as bass  # noqa: E402
import concourse.mybir as mybir  # noqa: E402
import concourse.tile as tile  # noqa: E402
from concourse import bacc  # noqa: E402
from concourse.bass_utils import run_bass_kernel_spmd  # noqa: E402

f32 = mybir.dt.float32
f32r = mybir.dt.float32r
bf16 = mybir.dt.bfloat16
P = 128


def build_core_program(D=1024, TOK=2048, NH=4, num_devices=8, warm_n=12):
    DH = 64              # head dim
    KD = D // P          # hidden-dim 128-chunks (8)
    NQ = TOK // 512      # 512-wide q chunks (4)
    NT = TOK // P        # 128-wide token chunks (16)
    DC = NH * DH         # per-core head dims (256)
    MQ = DC // P         # 128-row chunks of qT/kT/oT (2)
    HPC = P // DH        # heads per 128-row chunk (2)
    G = NT // 2          # k-chunk pairs per head (8)

    nc = bacc.Bacc("TRN2", target_bir_lowering=False, debug=False,
                   num_devices=num_devices)

    xT_d = nc.declare_dram_parameter("xT", [D, TOK], bf16, isOutput=False)
    wq_d = nc.declare_dram_parameter("wq", [D, DC], bf16, isOutput=False)
    wk_d = nc.declare_dram_parameter("wk", [D, DC], bf16, isOutput=False)
    wv_d = nc.declare_dram_parameter("wv", [D, DC], bf16, isOutput=False)
    wo_d = nc.declare_dram_parameter("wo", [DC, D], bf16, isOutput=False)
    bq_d = nc.declare_dram_parameter("bq", [P, MQ], f32, isOutput=False)
    id_d = nc.declare_dram_parameter("ident", [P, P], bf16, isOutput=False)
    out_d = nc.declare_dram_parameter("out", [TOK, D], bf16, isOutput=True)

    with tile.TileContext(nc) as tc, ExitStack() as ctx:
        persist = ctx.enter_context(tc.tile_pool(name="persist", bufs=1))
        work = ctx.enter_context(tc.tile_pool(name="work", bufs=1))
        psc = ctx.enter_context(tc.tile_pool(name="psc", bufs=2, space="PSUM"))
        pav = ctx.enter_context(tc.tile_pool(name="pav", bufs=2, space="PSUM"))
        pacc = ctx.enter_context(tc.tile_pool(name="pacc", bufs=2, space="PSUM"))

        # ---- PE warmup: ramp the p-state while the first DMAs land -----
        warm = persist.tile([P, P], f32)
        nc.vector.memset(warm[:], 0.0)
        warm_ps = pacc.tile([P, 512], f32, tag="acc", name="warm_ps")
        for _ in range(warm_n):
            nc.tensor.matmul(warm_ps[:, 0:P], warm[:], warm[:],
                             start=True, stop=True)

        zbias = persist.tile([P, 1], f32)
        nc.vector.memset(zbias[:], 0.0)

        # ---- phase A: DMAs.  First-need order: wq + xT n0 + bq (first
        # projection), wk (kT), wv (v blocks), then the rest.  Queue issue
        # costs: SP/ACT seq ~1.26us, Pool engine ~1.04us, DVE ~0.67us.
        xT_sb = persist.tile([P, KD, TOK], bf16)
        wq_sb = persist.tile([P, KD, DC], bf16)
        wk_sb = work.tile([P, KD, DC], bf16)
        wv_sb = work.tile([P, KD, DC], bf16)
        wo_sb = persist.tile([P, MQ, D], bf16)
        bq_sb = persist.tile([P, MQ], f32)
        ident = persist.tile([P, P], bf16)

        def xchunk(q, n, ko):
            q.dma_start(xT_sb[:, ko, n * 512:(n + 1) * 512],
                        xT_d[ko * P:(ko + 1) * P, n * 512:(n + 1) * 512])

        # SP: wq, wk, xT n0 ko6-7, then n1-n3 ko0-2 (+n2/n3 ko7)
        nc.sync.dma_start(wq_sb[:], wq_d.rearrange("(ko ki) n -> ki ko n", ki=P))
        xchunk(nc.sync, 0, 6)
        xchunk(nc.sync, 0, 7)
        nc.sync.dma_start(wk_sb[:], wk_d.rearrange("(ko ki) n -> ki ko n", ki=P))
        for n in range(1, NQ):
            for ko in range(3):
                xchunk(nc.sync, n, ko)
            if n >= 2:
                xchunk(nc.sync, n, 7)
        # Pool: xT n0 ko0-3, n1 ko3, wv, n1 ko4-7, ident, n2 ko3-5, wo,
        # n3 ko3-5
        for ko in range(4):
            xchunk(nc.gpsimd, 0, ko)
        xchunk(nc.gpsimd, 1, 3)
        nc.gpsimd.dma_start(wv_sb[:], wv_d.rearrange("(ko ki) n -> ki ko n", ki=P))
        for ko in range(4, KD):
            xchunk(nc.gpsimd, 1, ko)
        nc.gpsimd.dma_start(ident[:], id_d[:])
        for ko in range(3, 6):
            xchunk(nc.gpsimd, 2, ko)
        nc.gpsimd.dma_start(wo_sb[:], wo_d.rearrange("(mo mi) n -> mi mo n", mi=P))
        for ko in range(3, 6):
            xchunk(nc.gpsimd, 3, ko)
        # ACT: bq, xT n0 ko4-5, n2/n3 ko6 (then ACT is free for exps)
        nc.scalar.dma_start(bq_sb[:], bq_d[:])
        xchunk(nc.scalar, 0, 4)
        xchunk(nc.scalar, 0, 5)
        xchunk(nc.scalar, 2, 6)
        xchunk(nc.scalar, 3, 6)

        qT_sb = persist.tile([P, MQ, TOK], f32r)
        kT_sb = persist.tile([P, MQ, TOK], f32r)
        v_sb = persist.tile([P, NT, NH, DH + 1], bf16)
        oT_sb = persist.tile([P, MQ, TOK], bf16)
        onesc = persist.tile([P, 1], f32)
        nc.vector.memset(onesc[:], 1.0)
        nc.vector.tensor_copy(
            v_sb[:, :, :, DH:DH + 1],
            onesc[:, None, :].to_broadcast([P, NT, NH, 1]))

        # preload the Exp table while ACT is idle (zbias: no const-DMA dep)
        wtab = work.tile([1, 1], f32)
        nc.vector.memset(wtab[:], 0.0)
        wtab2 = work.tile([1, 1], f32)
        nc.scalar.activation(wtab2[:], wtab[:],
                             mybir.ActivationFunctionType.Exp,
                             bias=zbias[0:1, :], scale=0.125)

        def proj_block(w_sb, b_sb, t_sb, m, n, order=None):
            ps = pacc.tile([P, 512], f32, tag="acc", name="ps")
            kos = list(order) if order is not None else list(range(KD))
            for i, ko in enumerate(kos):
                nc.tensor.matmul(
                    ps[:], w_sb[:, ko, m * P:(m + 1) * P],
                    xT_sb[:, ko, n * 512:(n + 1) * 512],
                    start=(i == 0), stop=(i == KD - 1))
            if b_sb is None:
                nc.vector.tensor_copy(t_sb[:, m, n * 512:(n + 1) * 512], ps[:])
            else:
                nc.vector.tensor_tensor(
                    t_sb[:, m, n * 512:(n + 1) * 512], ps[:],
                    b_sb[:, m:m + 1].to_broadcast([P, 512]),
                    mybir.AluOpType.add)

        def v_block(t):
            ps = pacc.tile([P, DC], f32, tag="acc", name="vps")
            for ko in range(KD):
                nc.tensor.matmul(
                    ps[:], xT_sb[:, ko, t * P:(t + 1) * P], wv_sb[:, ko, :],
                    start=(ko == 0), stop=(ko == KD - 1))
            nc.vector.tensor_copy(
                v_sb[:, t, :, 0:DH],
                ps.rearrange("p (h d) -> p h d", h=NH))

        def emit_scores(pair, n, g, scs):
            qs = slice(n * 512, (n + 1) * 512)
            # Priority boost: when the psc ring frees a slot, the next
            # scores matmul should preempt queued fillers on the PE so the
            # exp stream (the pacer) never starves behind filler bursts.
            with tc.high_priority(offset=400):
                for j in range(2):
                    kk = g * 2 + j
                    for h in pair:
                        hm = h // HPC
                        hr = (h % HPC) * DH
                        nc.tensor.matmul(
                            scs[h][:, j, :],
                            kT_sb[hr:hr + DH, hm, kk * P:(kk + 1) * P],
                            qT_sb[hr:hr + DH, hm, qs],
                            start=True, stop=True)

        def emit_av(pair, g, avs, exs, qs=range(4)):
            # start=False always: the av bank is pre-zeroed by DVE memset, so
            # the four per-qsub accumulation regions in one bank never issue a
            # bank-wide zero (HW start flag marks the whole 2KB zero region).
            for h in pair:
                for j in range(2):
                    kk = g * 2 + j
                    for q in qs:
                        nc.tensor.matmul(
                            avs[h][:, q, :],
                            exs[h][:, j, q * P:(q + 1) * P],
                            v_sb[:, kk, h, :],
                            start=False,
                            stop=(g == G - 1 and j == 1),
                            skip_group_check=True)

        def emit_normalize(pair, avs, opair):
            # DVE priority boost: normalize gates the transpose/oproj chain;
            # it must preempt queued output copies on the DVE.
            with tc.high_priority(offset=400):
                for h in pair:
                    hr = (h % HPC) * DH
                    rec = work.tile([P, NH, 1], f32, tag=f"rec{h % HPC}",
                                    bufs=2, name="rec")
                    nc.vector.reciprocal(rec[:], avs[h][:, :, DH:DH + 1])
                    nc.vector.tensor_tensor(
                        opair[:, :, hr:hr + DH], avs[h][:, :, 0:DH],
                        rec.to_broadcast([P, NH, DH]),
                        mybir.AluOpType.mult)

        def emit_transpose(pi, n, q, opair, pool=None, tag="av"):
            tp = (pool or pav).tile([P, P], bf16, tag=tag, name="tp")
            nc.tensor.transpose(tp[:], opair[:, q, :], ident[:])
            nc.vector.tensor_copy(
                oT_sb[:, pi, n * 512 + q * P:n * 512 + (q + 1) * P], tp[:])

        _ou_state = {}  # tok -> (ou tile, halves done)

        def oproj_tile(n, t, nn, copy_eng=None):
            tok = n * 4 + t
            ns = slice(nn * 512, (nn + 1) * 512)
            op = pacc.tile([P, 512], f32, tag="acc", name="op")
            for m in range(MQ):
                nc.tensor.matmul(
                    op[:], oT_sb[:, m, tok * P:(tok + 1) * P],
                    wo_sb[:, m, ns],
                    start=(m == 0), stop=(m == MQ - 1))
            if tok not in _ou_state:
                _ou_state[tok] = [work.tile([P, D], bf16, tag="out", bufs=3,
                                            name="ou"), 0]
            ou, done = _ou_state[tok]
            # Pool/gpsimd cannot read PSUM (walrus backend rejects), so all
            # mid-stream output copies go to DVE.
            eng = copy_eng if copy_eng is not None else nc.vector
            if eng is nc.scalar:
                nc.scalar.copy(ou[:, ns], op[:])
            else:
                eng.tensor_copy(ou[:, ns], op[:])
            _ou_state[tok][1] = done + 1
            if _ou_state[tok][1] == 2:
                nc.sync.dma_start(out_d[tok * P:(tok + 1) * P, :], ou[:])
                del _ou_state[tok]

        # ---- phase B front: first-scores critical path only.  qT first
        # (scores need its full 512-token moving operand), then kT in two
        # half-width blocks: scores(g0) reads only k-tokens 0..255, so the
        # first exp launches before the second half projects. ------------
        ARRIVAL = (0, 1, 4, 2, 5, 6, 3, 7)   # first-chunk DMA landing order

        def proj_half(w_sb, b_sb, t_sb, m, lo):
            ps = pacc.tile([P, 256], f32, tag="acc", name="ph")
            for i, ko in enumerate(ARRIVAL):
                nc.tensor.matmul(
                    ps[:], w_sb[:, ko, m * P:(m + 1) * P],
                    xT_sb[:, ko, lo:lo + 256],
                    start=(i == 0), stop=(i == KD - 1))
            if b_sb is None:
                nc.vector.tensor_copy(t_sb[:, m, lo:lo + 256], ps[:])
            else:
                nc.vector.tensor_tensor(
                    t_sb[:, m, lo:lo + 256], ps[:],
                    b_sb[:, m:m + 1].to_broadcast([P, 256]),
                    mybir.AluOpType.add)

        proj_half(wq_sb, bq_sb, qT_sb, 0, 0)
        proj_half(wq_sb, bq_sb, qT_sb, 0, 256)
        proj_half(wk_sb, None, kT_sb, 0, 0)

        # ---- windowed pipeline: 8 windows = (pair0 n0..3, pair1 n0..3).
        # Window w runs scores+exp for its own (pi, n) while the PE drains
        # the AV matmuls of window w-1 (the exp tiles of w-1 are all ready,
        # so AV never head-of-line-blocks the queue).
        W = [(pi, n) for pi in range(2) for n in range(NQ)]
        F = [[[] for _ in range(G)] for _ in range(8)]

        def add(w, g, fn, *a):
            F[w][g].append((fn, a))

        # projection blocks split into two-slot halves so a 1.7us PE burst
        # never outruns the 2-deep psc cushion and stalls ACT
        def staged_proj(w, g, w_sb, b_sb, t_sb, m, n):
            box = {}

            def p0():
                ps = pacc.tile([P, 512], f32, tag="acc", name="ps")
                box["ps"] = ps
                for ko in range(KD // 2):
                    nc.tensor.matmul(
                        ps[:], w_sb[:, ko, m * P:(m + 1) * P],
                        xT_sb[:, ko, n * 512:(n + 1) * 512],
                        start=(ko == 0), stop=False)

            def p1():
                ps = box["ps"]
                for ko in range(KD // 2, KD):
                    nc.tensor.matmul(
                        ps[:], w_sb[:, ko, m * P:(m + 1) * P],
                        xT_sb[:, ko, n * 512:(n + 1) * 512],
                        start=False, stop=(ko == KD - 1))
                if b_sb is None:
                    nc.vector.tensor_copy(
                        t_sb[:, m, n * 512:(n + 1) * 512], ps[:])
                else:
                    nc.vector.tensor_tensor(
                        t_sb[:, m, n * 512:(n + 1) * 512], ps[:],
                        b_sb[:, m:m + 1].to_broadcast([P, 512]),
                        mybir.AluOpType.add)

            add(w, g, p0)
            add(w, g + 1, p1)

        # window 0: kT m0 h1 (scores g1 needs it), kT b1-b3 staged,
        # v0..v5 1/slot, qT m0 n1 and n2
        add(0, 0, proj_half, wk_sb, None, kT_sb, 0, 256)
        staged_proj(0, 0, wk_sb, None, kT_sb, 0, 1)
        staged_proj(0, 2, wk_sb, None, kT_sb, 0, 2)
        staged_proj(0, 4, wk_sb, None, kT_sb, 0, 3)
        for g in range(6):
            add(0, g, v_block, g)
        add(0, 6, proj_block, wq_sb, bq_sb, qT_sb, 0, 1)
        add(0, 7, proj_block, wq_sb, bq_sb, qT_sb, 0, 2)
        # window 1: v6..v15 paced ~1.25/slot (AV(w0, g) at slot g reads
        # v(2g..2g+1); each v lands >=1 slot ahead of its drain)
        W1V = {0: (6, 7), 1: (8,), 2: (9, 10), 3: (11,), 4: (12, 13),
               5: (14,), 6: (15,)}
        for g, vs in W1V.items():
            for t in vs:
                add(1, g, v_block, t)
        # window 2: kT m1 b0/b1, qT m0 n3
        staged_proj(2, 1, wk_sb, None, kT_sb, 1, 0)
        staged_proj(2, 3, wk_sb, None, kT_sb, 1, 1)
        staged_proj(2, 5, wq_sb, bq_sb, qT_sb, 0, 3)
        # window 3: kT m1 b2/b3, qT m1 n0
        staged_proj(3, 1, wk_sb, None, kT_sb, 1, 2)
        staged_proj(3, 3, wk_sb, None, kT_sb, 1, 3)
        staged_proj(3, 5, wq_sb, bq_sb, qT_sb, 1, 0)
        # window 4: qT m1 n1 and n2 (w4 has drain-only slack)
        staged_proj(4, 1, wq_sb, bq_sb, qT_sb, 1, 1)
        staged_proj(4, 4, wq_sb, bq_sb, qT_sb, 1, 2)
        # window 5 late slots: last qT stage (deadline: w7 g0)
        staged_proj(5, 5, wq_sb, bq_sb, qT_sb, 1, 3)

        def alloc_avs(pair):
            avs = {h: pav.tile([P, NH, DH + 1], f32, tag="av",
                               name=f"av{h}") for h in pair}
            for h in pair:
                nc.vector.memset(avs[h][:], 0.0)
            return avs

        def normalize_stream(st):
            pair, avs = st["pair"], st["avs"]
            opair = work.tile([P, 4, P], bf16, tag="opair", bufs=2,
                              name="opair")
            emit_normalize(pair, avs, opair)
            return (st["pi"], st["n"], opair)

        def drain_group(st, g):
            emit_av(st["pair"], g, st["avs"], st["exs"][g])

        # accel windows w>=5: drain prev AV 4+4 groups at slots 0-1,
        # normalize prev after slot-1 drains, transposes slot 2, oproj
        # spread over slots 2-5.
        ACCEL_DRAIN = {0: (0, 1), 1: (2, 3), 2: (4, 5), 3: (6, 7)}
        OPROJ_SLOT = {4: ((0, 0), (0, 1)),
                      5: ((1, 0), (1, 1), (2, 0)),
                      6: ((2, 1), (3, 0), (3, 1))}
        OWN_DRAIN = {5: (0, 1), 6: (2, 3), 7: (4, 5, 6)}

        pend_tp = []        # transposes to emit at the next window's g0
        prev_st = None      # stream of window w-1 awaiting AV
        for w in range(8):
            pi, n = W[w]
            pair = [pi * HPC + i for i in range(HPC)]
            cur_st = {"pi": pi, "n": n, "pair": pair, "exs": []}
            accel = (w >= 5)
            norm_tp = None
            for g in range(G):
                scs = {h: psc.tile([P, 2, 512], f32, tag="sc",
                                   name=f"sc{h}") for h in pair}
                emit_scores(pair, n, g, scs)
                exs = {}
                for h in pair:
                    ex = work.tile([P, 2, 512], bf16, tag=f"ex{h % HPC}",
                                   bufs=13, name="ex")
                    nc.scalar.activation(
                        ex[:], scs[h][:],
                        mybir.ActivationFunctionType.Exp,
                        bias=zbias[:, :], scale=0.125)
                    exs[h] = ex
                cur_st["exs"].append(exs)
                if g == 0:
                    for src_pi, src_n, src_op in pend_tp:
                        for q in range(4):
                            emit_transpose(src_pi, src_n, q, src_op)
                    pend_tp = []
                    if prev_st is not None:
                        prev_st["avs"] = alloc_avs(prev_st["pair"])
                for fn, a in F[w][g]:
                    fn(*a)
                if not accel:
                    if prev_st is not None:
                        drain_group(prev_st, g)
                    continue
                # accel windows
                if w == 7 and g in OWN_DRAIN:
                    # drain own groups g0..g6 while exps land (lag 2)
                    for gg in OWN_DRAIN[g]:
                        emit_av(pair, gg, cur_st["avs"], cur_st["exs"][gg])
                if g in ACCEL_DRAIN:
                    for gg in ACCEL_DRAIN[g]:
                        drain_group(prev_st, gg)
                    if g == 3:
                        norm_tp = normalize_stream(prev_st)
                elif g == 4:
                    src_pi, src_n, src_op = norm_tp
                    for q in range(4):
                        emit_transpose(src_pi, src_n, q, src_op)
                    if w == 7:
                        # own avs reuse the 2-slot psum ring: alloc only
                        # after w6's transposes so the ring handoff order is
                        # w6-avs -> tps -> own-avs (each freed quickly), not
                        # tps stuck behind the own-avs' whole lifetime.
                        cur_st["avs"] = alloc_avs(pair)
                if g in OPROJ_SLOT:
                    n_o = w - 5
                    for t, nn in OPROJ_SLOT[g]:
                        oproj_tile(n_o, t, nn)
            if w == 7:
                tail_st = cur_st
            elif accel:
                prev_st = cur_st
            else:
                if prev_st is not None:
                    pend_tp.append(normalize_stream(prev_st))
                prev_st = cur_st

        # ---- tail: per-qsub pipeline on window 7 (pi1, n3).  Drain g7's
        # q-slice, normalize that qsub, transpose, project tokens 12+q with
        # copies alternating ACT/DVE and half-tile out DMAs on SP/Pool.
        st = tail_st
        avs = st["avs"]
        opair = work.tile([P, 4, P], bf16, tag="opair", bufs=2, name="opair")

        def tail_norm(q):
            for h in st["pair"]:
                hr = (h % HPC) * DH
                rec = work.tile([P, 1, 1], f32, tag=f"rec{h % HPC}", bufs=2,
                                name="rec")
                nc.vector.reciprocal(rec[:], avs[h][:, q:q + 1, DH:DH + 1])
                nc.vector.tensor_tensor(
                    opair[:, q, hr:hr + DH], avs[h][:, q, 0:DH],
                    rec[:, 0, :].to_broadcast([P, DH]),
                    mybir.AluOpType.mult)

        def tail_op(q, nn):
            tok = 12 + q
            ns = slice(nn * 512, (nn + 1) * 512)
            # late-tail nn1 ops use the freed av ring so the two psum rings
            # interleave and the 2-deep acc ring's copy latency is hidden
            pool, tag = (pav, "av") if (q >= 2 and nn == 1) else (pacc, "acc")
            op = pool.tile([P, 512], f32, tag=tag, name="op")
            for m in range(MQ):
                nc.tensor.matmul(
                    op[:], oT_sb[:, m, tok * P:(tok + 1) * P],
                    wo_sb[:, m, ns],
                    start=(m == 0), stop=(m == MQ - 1))
            ou = work.tile([P, 512], bf16, tag="outh", bufs=8, name="ouh")
            if nn == 0:
                nc.scalar.copy(ou[:], op[:])
                nc.sync.dma_start(out_d[tok * P:(tok + 1) * P, ns], ou[:])
            else:
                nc.vector.tensor_copy(ou[:], op[:])
                dq = nc.scalar if q == 3 else nc.gpsimd
                dq.dma_start(out_d[tok * P:(tok + 1) * P, ns], ou[:])

        def tail_av(q):
            emit_av(st["pair"], G - 1, avs, st["exs"][G - 1], qs=(q,))

        # software pipeline: drain q+2 and normalize q+1 while q's
        # transpose/oproj chain runs, so the in-order PE queue never
        # head-of-line blocks on DVE latency.
        tail_av(0)
        tail_norm(0)
        tail_av(1)
        tail_norm(1)
        for q in range(4):
            # tail tps go in the retired scores ring: the av ring's slots
            # are held by the own-avs until all four normalizes have read
            # them, which would serialize the tail.
            emit_transpose(st["pi"], st["n"], q, opair, pool=psc, tag="sc")
            if q + 2 < 4:
                tail_av(q + 2)
                tail_norm(q + 2)
            if q == 3:
                # kernel-ending piece first so its copy/DMA chain starts
                # as early as possible
                tail_op(q, 1)
                tail_op(q, 0)
            else:
                tail_op(q, 0)
                tail_op(q, 1)
    return nc


_CACHE = {}
LAST_RESULTS = None


def _get_compiled():
    if "nc" not in _CACHE:
        nc = build_core_program()
        nc.compile()
        _CACHE["nc"] = nc
    return _CACHE["nc"]


def kernel(x, wq, bq, wk, bk, wv, bv, wo, bo):
    global LAST_RESULTS
    import ml_dtypes
    bft = ml_dtypes.bfloat16
    x = np.asarray(x, np.float32)
    wq, bq = np.asarray(wq, np.float32), np.asarray(bq, np.float32)
    wk = np.asarray(wk, np.float32)
    wv, bv = np.asarray(wv, np.float32), np.asarray(bv, np.float32)
    wo, bo = np.asarray(wo, np.float32), np.asarray(bo, np.float32)
    B, TOK, D = x.shape          # (2, 2048, 1024)
    NH, DH = 4, 64               # heads per core, head dim
    DC = NH * DH                 # 256
    MQ = DC // P                 # 2

    nc = _get_compiled()

    ident = np.eye(P, dtype=np.float32)

    in_maps = []
    for c in range(8):
        b, hg = c // 4, c % 4
        sl = slice(hg * DC, (hg + 1) * DC)
        in_maps.append({
            "xT": np.ascontiguousarray(x[b].T).astype(bft),
            "wq": np.ascontiguousarray(wq[:, sl]).astype(bft),
            "wk": np.ascontiguousarray(wk[:, sl]).astype(bft),
            "wv": np.ascontiguousarray(wv[:, sl]).astype(bft),
            "wo": np.ascontiguousarray(wo[sl, :]).astype(bft),
            "bq": np.ascontiguousarray(bq[sl].reshape(MQ, P).T),
            "ident": ident.astype(bft),
        })

    trace = os.environ.get("KERNEL_TRACE", "0") == "1"
    res = run_bass_kernel_spmd(nc, in_maps, core_ids=list(range(8)),
                               trace=trace)
    LAST_RESULTS = res
    outs = [np.asarray(res.results[c]["out"], dtype=np.float32)
            for c in range(8)]
    # bv passes through softmax: softmax@(v+bv) = softmax@v + bv, so the
    # device drops bv and the host folds bv@wo into the output bias.
    bo_eff = (bo.astype(np.float64) + bv.astype(np.float64)
              @ wo.astype(np.float64)).astype(np.float32)
    y = (np.stack([sum(outs[0:4]), sum(outs[4:8])], axis=0)
         + bo_eff[None, None, :])
    return np.ascontiguousarray(y, dtype=np.float32)
